# revision 31
# baseline (speedup 1.0000x reference)
"""Trainium2 Bass kernel for nn_LogisticModel.

Computes, for each batch row b:
    logp[b] = sum_t Normal(x_t - 0.9*x_{t-1} - sigmoid(s_t), 0.1).logpdf(0)
            = -0.5/0.01 * sum_t resid_t^2 + T * (-ln(0.1) - 0.5*ln(2*pi))
with x_{-1} = 0.  Pure elementwise + row reduction; sharded by batch rows
across 8 NeuronCores (512 rows per core).

The tolerance budget (rel err 2e-2 on a |logp| ~ 8.5e5 output) is enormous,
so inputs are downcast during the host-side shard step: s -> fp8 e4m3 (only
the ACT engine reads it, for sigmoid) and x -> bf16 (so DVE tensor ops get
the packed-16-bit fast modes).  HBM traffic per core drops 32 -> 12.6 MiB.

Per-chunk engine split (v = 0.9*x_prev - (x - sigmoid(s)) = -resid).
GpSimd compute is avoided entirely: it shares SBUF ports with DVE, and
measured traces show DVE tensor ops stall up to 20x while GpSimd streams.
tensor_tensor_reduce crashes the exec unit on this runtime, so DVE's
share of the reduce uses bn_stats:
    ACT : sigma = sigmoid(s)            (fp8 in, bf16 out)
    DVE : w = x - sigma                 (TT, bf16, 2x mode)
    DVE : pp = 0.9 * x_prev             (TS, bf16, 4x mode)
    DVE : v = pp - w                    (TT, bf16, 2x, in-place into pp)
    ACT : Square(v[:, :C]) accum        (fused square+reduce, ~78% of cols)
    DVE : bn_stats per 512-col block of v[:, C:]; sum v^2 recovered in the
          tail as M2_e + M2_o + 256*(mean_e^2 + mean_o^2) per block.

Chunk widths ramp small -> large -> small so the ACT/DVE pipeline fills
quickly after the first small DMA and drains quickly after the last one.

Self-contained: hardcodes B=4096, T=8192.
"""

import math
import sys

import ml_dtypes
import numpy as np

sys.path.insert(0, "/opt/trn_rl_repo")

import concourse.bacc as bacc  # noqa: E402
import concourse.tile as tile  # noqa: E402
from concourse import mybir  # noqa: E402
from concourse.bass_utils import run_bass_kernel_spmd  # noqa: E402

GAIN = 1.0
DECAY = 0.9
NOISE = 0.1
LOG_2PI = math.log(2.0 * math.pi)

B, T = 4096, 8192
N_CORES = 8
ROWS_PER_CORE = B // N_CORES          # 512
P = 128                               # SBUF partitions
N_GROUP = ROWS_PER_CORE // P          # 4 row-groups per core
BLK = 512                             # bn_stats hardware block limit

C1 = -0.5 / (NOISE * NOISE)                      # -50.0
C2 = T * (-math.log(NOISE) - 0.5 * LOG_2PI)      # per-row additive constant

# Per-group chunk plans: (width, act_cols) pairs; act_cols go through the
# ACT Square path, the rest through DVE bn_stats ((w - cb) % BLK == 0).
# First group starts small (fast pipeline fill), last group ends small
# (fast drain).  Aggregate ACT fraction ~0.78: bn_stats costs ~1.16ns/elem
# on DVE vs 0.83 on ACT, so ACT leans heavier.
_PLAN_A = [(1024, 512), (3072, 2560), (4096, 3584)]
_PLAN_B = [(1024, 512), (3072, 2560), (4096, 3072)]
_PLAN_TAIL = [(4096, 3072), (3072, 2560), (1024, 512)]

_cache = {}


def _build(bufs=4, xbufs=3):
    """Build and schedule the per-core Tile kernel (same program on all 8)."""
    nc = bacc.Bacc("TRN2", target_bir_lowering=False, debug=False,
                   num_devices=N_CORES)
    f32 = mybir.dt.float32
    bf16 = mybir.dt.bfloat16
    fp8 = mybir.dt.float8e4
    s_d = nc.dram_tensor("s", [ROWS_PER_CORE, T], fp8, kind="ExternalInput").ap()
    x_d = nc.dram_tensor("x", [ROWS_PER_CORE, T], bf16, kind="ExternalInput").ap()
    o_d = nc.dram_tensor("o", [P, N_GROUP], f32, kind="ExternalOutput").ap()

    Alu = mybir.AluOpType
    Act = mybir.ActivationFunctionType

    plans = [list(_PLAN_A), list(_PLAN_A), list(_PLAN_B), list(_PLAN_TAIL)]
    for pl in plans:
        assert sum(w for w, _ in pl) == T
        assert all((w - cb) % BLK == 0 for w, cb in pl)
    max_chunks = max(len(pl) for pl in plans)
    max_blk = max(sum((w - cb) // BLK for w, cb in pl) for pl in plans)

    with tile.TileContext(nc) as tc:
        with (
            tc.tile_pool(name="xp", bufs=xbufs) as xp,
            tc.tile_pool(name="io", bufs=bufs) as io,
            tc.tile_pool(name="accp", bufs=1) as accp,
        ):
            acc = accp.tile([P, N_GROUP, max_chunks], f32)    # ACT partials
            stats = accp.tile([P, N_GROUP, max_blk, 6], f32)  # bn_stats out
            me = accp.tile([P, N_GROUP, max_blk, 1], f32)
            mo = accp.tile([P, N_GROUP, max_blk, 1], f32)
            m2 = accp.tile([P, N_GROUP, max_blk, 1], f32)
            bs = accp.tile([P, N_GROUP, max_blk, 1], f32)
            junk = accp.tile([P, max(cb for pl in plans for _, cb in pl)],
                             bf16, name="junk")  # ACT Square scratch
            t_act = accp.tile([P, N_GROUP], f32)
            t_dve = accp.tile([P, N_GROUP], f32)
            logp = accp.tile([P, N_GROUP], f32)
            nc.vector.memset(acc[:], 0.0)
            nc.vector.memset(stats[:], 0.0)

            for g in range(N_GROUP):
                rows = slice(g * P, (g + 1) * P)
                plan = plans[g]
                # Whole-row x tile with one zero pad column at the front so
                # x_prev is just a shifted view (no overlap re-read).
                xx = xp.tile([P, T + 1], bf16, tag="xx")
                nc.vector.memset(xx[:, 0:1], 0.0)
                # All s loads before x loads: sigmoid (the ACT critical
                # path) never queues behind a bulkier x transfer, so ACT
                # starts chunk j+1's sigmoid while x_j is still landing.
                s_tiles = []
                col = 0
                for j, (w_, cb) in enumerate(plan):
                    s_t = io.tile([P, w_], fp8, tag="s", name=f"s{g}_{j}")
                    s_tiles.append(s_t)
                    nc.sync.dma_start(out=s_t[:], in_=s_d[rows, col:col + w_])
                    col += w_
                col = 0
                for j, (w_, cb) in enumerate(plan):
                    nc.sync.dma_start(out=xx[:, col + 1:col + w_ + 1],
                                      in_=x_d[rows, col:col + w_])
                    col += w_

                blk_i = 0
                col = 0
                for j, (w_, cb) in enumerate(plan):
                    s_t = s_tiles[j]
                    sig = io.tile([P, w_], bf16, tag="sig")
                    w_t = io.tile([P, w_], bf16, tag="w")
                    pp = io.tile([P, w_], bf16, tag="pp")

                    # sigma = sigmoid(GAIN * s)
                    nc.scalar.activation(out=sig[:], in_=s_t[:],
                                         func=Act.Sigmoid, scale=GAIN)
                    # w = x - sigma  (TT, bf16 -> 2x mode)
                    nc.vector.tensor_sub(w_t[:], xx[:, col + 1:col + w_ + 1],
                                         sig[:])
                    # pp = 0.9 * x_prev  (TS, bf16 -> 4x mode)
                    nc.vector.tensor_scalar(out=pp[:], in0=xx[:, col:col + w_],
                                            scalar1=DECAY, scalar2=None,
                                            op0=Alu.mult)
                    # v = pp - w = -resid, computed in-place into pp
                    # (elementwise, identical APs: each element is read
                    # before it is overwritten)
                    nc.vector.tensor_sub(pp[:], pp[:], w_t[:])
                    # acc[:, g, j] = sum v[:, :cb]^2 on ACT
                    nc.scalar.activation(out=junk[:, 0:cb], in_=pp[:, 0:cb],
                                         func=Act.Square,
                                         accum_out=acc[:, g, j:j + 1])
                    # per-512 stats of v[:, cb:] on DVE
                    for bkt in range((w_ - cb) // BLK):
                        c0 = cb + bkt * BLK
                        nc.vector.bn_stats(
                            stats[:, g, blk_i, :], pp[:, c0:c0 + BLK])
                        blk_i += 1
                    col += w_

            # tail: recover sum v^2 from bn_stats, fold with ACT partials
            nc.vector.tensor_mul(me[:], stats[:, :, :, 1:2],
                                 stats[:, :, :, 1:2])
            nc.vector.tensor_mul(mo[:], stats[:, :, :, 4:5],
                                 stats[:, :, :, 4:5])
            nc.vector.tensor_add(me[:], me[:], mo[:])
            nc.vector.tensor_add(m2[:], stats[:, :, :, 2:3],
                                 stats[:, :, :, 5:6])
            # bs = (BLK/2) * (mean_e^2 + mean_o^2) + (M2_e + M2_o)
            nc.vector.scalar_tensor_tensor(
                out=bs[:], in0=me[:], scalar=BLK / 2, in1=m2[:],
                op0=Alu.mult, op1=Alu.add)
            nc.vector.tensor_reduce(
                out=t_dve[:], in_=bs[:, :, :, 0],
                axis=mybir.AxisListType.X, op=Alu.add)
            nc.vector.tensor_reduce(
                out=t_act[:], in_=acc[:],
                axis=mybir.AxisListType.X, op=Alu.add)
            nc.vector.tensor_add(logp[:], t_dve[:], t_act[:])
            nc.vector.tensor_scalar(
                out=logp[:], in0=logp[:], scalar1=C1, scalar2=C2,
                op0=Alu.mult, op1=Alu.add,
            )
            nc.sync.dma_start(out=o_d[:], in_=logp[:])

    nc.compile()
    return nc


def _run(s, x, trace=False, **build_kwargs):
    key = tuple(sorted(build_kwargs.items()))
    if key not in _cache:
        _cache[key] = _build(**build_kwargs)
    nc = _cache[key]

    s8 = np.asarray(s, dtype=np.float32).astype(ml_dtypes.float8_e4m3)
    x16 = np.asarray(x, dtype=np.float32).astype(ml_dtypes.bfloat16)

    in_maps = []
    for k in range(N_CORES):
        r0 = k * ROWS_PER_CORE
        in_maps.append({
            "s": np.ascontiguousarray(s8[r0:r0 + ROWS_PER_CORE]),
            "x": np.ascontiguousarray(x16[r0:r0 + ROWS_PER_CORE]),
        })

    res = run_bass_kernel_spmd(nc, in_maps, list(range(N_CORES)), trace=trace)

    out = np.empty((B,), dtype=np.float32)
    for k in range(N_CORES):
        # o[p, g] holds the row g*P + p of this core's shard
        out[k * ROWS_PER_CORE:(k + 1) * ROWS_PER_CORE] = (
            np.asarray(res.results[k]["o"]).T.reshape(-1)
        )
    return out, res


def kernel(s, x):
    out, _ = _run(np.asarray(s, dtype=np.float32), np.asarray(x, dtype=np.float32))
    return out


if __name__ == "__main__":
    rng = np.random.default_rng(0)
    s = rng.standard_normal((B, T), dtype=np.float32)
    x = rng.standard_normal((B, T), dtype=np.float32)
    out = kernel(s, x)
    print(out.shape, out.dtype, out[:4])


# revision 32
# speedup vs baseline: 1.0159x; 1.0159x over previous
"""Trainium2 Bass kernel for nn_LogisticModel.

Computes, for each batch row b:
    logp[b] = sum_t Normal(x_t - 0.9*x_{t-1} - sigmoid(s_t), 0.1).logpdf(0)
            = -0.5/0.01 * sum_t resid_t^2 + T * (-ln(0.1) - 0.5*ln(2*pi))
with x_{-1} = 0.  Pure elementwise + row reduction; sharded by batch rows
across 8 NeuronCores (512 rows per core).

The tolerance budget (rel err 2e-2 on a |logp| ~ 8.5e5 output) is enormous,
so inputs are downcast during the host-side shard step: s -> fp8 e4m3 (only
the ACT engine reads it, for sigmoid) and x -> bf16 (so DVE tensor ops get
the packed-16-bit fast modes).  HBM traffic per core drops 32 -> 12.6 MiB.

Per-chunk engine split (v = 0.9*x_prev - (x - sigmoid(s)) = -resid).
GpSimd compute is avoided entirely: it shares SBUF ports with DVE, and
measured traces show DVE tensor ops stall up to 20x while GpSimd streams.
tensor_tensor_reduce crashes the exec unit on this runtime, so DVE's
share of the reduce uses bn_stats:
    ACT : sigma = sigmoid(s)            (fp8 in, bf16 out)
    DVE : w = x - sigma                 (TT, bf16, 2x mode)
    DVE : pp = 0.9 * x_prev             (TS, bf16, 4x mode)
    DVE : v = pp - w                    (TT, bf16, 2x, in-place into pp)
    ACT : Square(v[:, :C]) accum        (fused square+reduce, ~78% of cols)
    DVE : bn_stats per 512-col block of v[:, C:]; sum v^2 recovered in the
          tail as M2_e + M2_o + 256*(mean_e^2 + mean_o^2) per block.

Chunk widths ramp small -> large -> small so the ACT/DVE pipeline fills
quickly after the first small DMA and drains quickly after the last one.

Self-contained: hardcodes B=4096, T=8192.
"""

import math
import sys

import ml_dtypes
import numpy as np

sys.path.insert(0, "/opt/trn_rl_repo")

import concourse.bacc as bacc  # noqa: E402
import concourse.tile as tile  # noqa: E402
from concourse import mybir  # noqa: E402
from concourse.bass_utils import run_bass_kernel_spmd  # noqa: E402

GAIN = 1.0
DECAY = 0.9
NOISE = 0.1
LOG_2PI = math.log(2.0 * math.pi)

B, T = 4096, 8192
N_CORES = 8
ROWS_PER_CORE = B // N_CORES          # 512
P = 128                               # SBUF partitions
N_GROUP = ROWS_PER_CORE // P          # 4 row-groups per core
BLK = 512                             # bn_stats hardware block limit

C1 = -0.5 / (NOISE * NOISE)                      # -50.0
C2 = T * (-math.log(NOISE) - 0.5 * LOG_2PI)      # per-row additive constant

# Per-group chunk plans: (width, act_cols) pairs; act_cols go through the
# ACT Square path, the rest through DVE bn_stats ((w - cb) % BLK == 0).
# First group starts small (fast pipeline fill), last group ends small
# (fast drain).  Aggregate ACT fraction ~0.78: bn_stats costs ~1.16ns/elem
# on DVE vs 0.83 on ACT, so ACT leans heavier.
_PLAN_A = [(1024, 512), (3072, 2560), (4096, 3584)]
_PLAN_B = [(1024, 512), (3072, 2560), (4096, 3072)]
_PLAN_TAIL = [(4096, 3072), (3072, 2560), (1024, 512)]

_cache = {}


def _build(bufs=4, xbufs=3):
    """Build and schedule the per-core Tile kernel (same program on all 8)."""
    nc = bacc.Bacc("TRN2", target_bir_lowering=False, debug=False,
                   num_devices=N_CORES)
    f32 = mybir.dt.float32
    bf16 = mybir.dt.bfloat16
    fp8 = mybir.dt.float8e4
    s_d = nc.dram_tensor("s", [ROWS_PER_CORE, T], fp8, kind="ExternalInput").ap()
    x_d = nc.dram_tensor("x", [ROWS_PER_CORE, T], bf16, kind="ExternalInput").ap()
    o_d = nc.dram_tensor("o", [P, N_GROUP], f32, kind="ExternalOutput").ap()

    Alu = mybir.AluOpType
    Act = mybir.ActivationFunctionType

    plans = [list(_PLAN_A), list(_PLAN_A), list(_PLAN_B), list(_PLAN_TAIL)]
    for pl in plans:
        assert sum(w for w, _ in pl) == T
        assert all((w - cb) % BLK == 0 for w, cb in pl)
    max_chunks = max(len(pl) for pl in plans)
    max_blk = max(sum((w - cb) // BLK for w, cb in pl) for pl in plans)

    with tile.TileContext(nc) as tc:
        with (
            tc.tile_pool(name="xp", bufs=xbufs) as xp,
            tc.tile_pool(name="io", bufs=bufs) as io,
            tc.tile_pool(name="accp", bufs=1) as accp,
        ):
            acc = accp.tile([P, N_GROUP, max_chunks], f32)    # ACT partials
            stats = accp.tile([P, N_GROUP, max_blk, 6], f32)  # bn_stats out
            me = accp.tile([P, N_GROUP, max_blk, 1], f32)
            mo = accp.tile([P, N_GROUP, max_blk, 1], f32)
            m2 = accp.tile([P, N_GROUP, max_blk, 1], f32)
            bs = accp.tile([P, N_GROUP, max_blk, 1], f32)
            junk = accp.tile([P, max(cb for pl in plans for _, cb in pl)],
                             bf16, name="junk")  # ACT Square scratch
            t_act = accp.tile([P, N_GROUP], f32)
            t_dve = accp.tile([P, N_GROUP], f32)
            logp = accp.tile([P, N_GROUP], f32)
            nc.vector.memset(acc[:], 0.0)
            nc.vector.memset(stats[:], 0.0)

            for g in range(N_GROUP):
                rows = slice(g * P, (g + 1) * P)
                plan = plans[g]
                # Whole-row x tile with one zero pad column at the front so
                # x_prev is just a shifted view (no overlap re-read).
                xx = xp.tile([P, T + 1], bf16, tag="xx")
                nc.vector.memset(xx[:, 0:1], 0.0)
                blk_i = 0
                col = 0
                for j, (w_, cb) in enumerate(plan):
                    s_t = io.tile([P, w_], fp8, tag="s")
                    sig = io.tile([P, w_], bf16, tag="sig")
                    w_t = io.tile([P, w_], bf16, tag="w")
                    pp = io.tile([P, w_], bf16, tag="pp")

                    nc.sync.dma_start(out=s_t[:], in_=s_d[rows, col:col + w_])
                    nc.sync.dma_start(out=xx[:, col + 1:col + w_ + 1],
                                      in_=x_d[rows, col:col + w_])
                    # sigma = sigmoid(GAIN * s)
                    nc.scalar.activation(out=sig[:], in_=s_t[:],
                                         func=Act.Sigmoid, scale=GAIN)
                    # w = x - sigma  (TT, bf16 -> 2x mode)
                    nc.vector.tensor_sub(w_t[:], xx[:, col + 1:col + w_ + 1],
                                         sig[:])
                    # pp = 0.9 * x_prev  (TS, bf16 -> 4x mode)
                    nc.vector.tensor_scalar(out=pp[:], in0=xx[:, col:col + w_],
                                            scalar1=DECAY, scalar2=None,
                                            op0=Alu.mult)
                    # v = pp - w = -resid, computed in-place into pp
                    # (elementwise, identical APs: each element is read
                    # before it is overwritten)
                    nc.vector.tensor_sub(pp[:], pp[:], w_t[:])
                    # acc[:, g, j] = sum v[:, :cb]^2 on ACT
                    nc.scalar.activation(out=junk[:, 0:cb], in_=pp[:, 0:cb],
                                         func=Act.Square,
                                         accum_out=acc[:, g, j:j + 1])
                    # per-512 stats of v[:, cb:] on DVE
                    for bkt in range((w_ - cb) // BLK):
                        c0 = cb + bkt * BLK
                        nc.vector.bn_stats(
                            stats[:, g, blk_i, :], pp[:, c0:c0 + BLK])
                        blk_i += 1
                    col += w_

            # tail: recover sum v^2 from bn_stats, fold with ACT partials
            nc.vector.tensor_mul(me[:], stats[:, :, :, 1:2],
                                 stats[:, :, :, 1:2])
            nc.vector.tensor_mul(mo[:], stats[:, :, :, 4:5],
                                 stats[:, :, :, 4:5])
            nc.vector.tensor_add(me[:], me[:], mo[:])
            nc.vector.tensor_add(m2[:], stats[:, :, :, 2:3],
                                 stats[:, :, :, 5:6])
            # bs = (BLK/2) * (mean_e^2 + mean_o^2) + (M2_e + M2_o)
            nc.vector.scalar_tensor_tensor(
                out=bs[:], in0=me[:], scalar=BLK / 2, in1=m2[:],
                op0=Alu.mult, op1=Alu.add)
            nc.vector.tensor_reduce(
                out=t_dve[:], in_=bs[:, :, :, 0],
                axis=mybir.AxisListType.X, op=Alu.add)
            nc.vector.tensor_reduce(
                out=t_act[:], in_=acc[:],
                axis=mybir.AxisListType.X, op=Alu.add)
            nc.vector.tensor_add(logp[:], t_dve[:], t_act[:])
            nc.vector.tensor_scalar(
                out=logp[:], in0=logp[:], scalar1=C1, scalar2=C2,
                op0=Alu.mult, op1=Alu.add,
            )
            nc.sync.dma_start(out=o_d[:], in_=logp[:])

    nc.compile()
    return nc


def _run(s, x, trace=False, **build_kwargs):
    key = tuple(sorted(build_kwargs.items()))
    if key not in _cache:
        _cache[key] = _build(**build_kwargs)
    nc = _cache[key]

    s8 = np.asarray(s, dtype=np.float32).astype(ml_dtypes.float8_e4m3)
    x16 = np.asarray(x, dtype=np.float32).astype(ml_dtypes.bfloat16)

    in_maps = []
    for k in range(N_CORES):
        r0 = k * ROWS_PER_CORE
        in_maps.append({
            "s": np.ascontiguousarray(s8[r0:r0 + ROWS_PER_CORE]),
            "x": np.ascontiguousarray(x16[r0:r0 + ROWS_PER_CORE]),
        })

    res = run_bass_kernel_spmd(nc, in_maps, list(range(N_CORES)), trace=trace)

    out = np.empty((B,), dtype=np.float32)
    for k in range(N_CORES):
        # o[p, g] holds the row g*P + p of this core's shard
        out[k * ROWS_PER_CORE:(k + 1) * ROWS_PER_CORE] = (
            np.asarray(res.results[k]["o"]).T.reshape(-1)
        )
    return out, res


def kernel(s, x):
    out, _ = _run(np.asarray(s, dtype=np.float32), np.asarray(x, dtype=np.float32))
    return out


if __name__ == "__main__":
    rng = np.random.default_rng(0)
    s = rng.standard_normal((B, T), dtype=np.float32)
    x = rng.standard_normal((B, T), dtype=np.float32)
    out = kernel(s, x)
    print(out.shape, out.dtype, out[:4])
